# revision 13
# baseline (speedup 1.0000x reference)
"""MADPSNet MoE-routing kernel for 8 Trainium2 NeuronCores.

The reference computes every expert on the full stacked input and then
gathers one expert per agent.  The routing indices (laac_shallow /
laac_deep) are host-visible numpy values, so we do the routing on the
host: per agent we select the 4 weight matrices of its chosen experts
and run only the selected chain

    x[2048,256] @ W1[256,512] -> relu -> @ W2[512,256] -> relu
                -> @ W3[256,512] -> relu -> @ W4[512,128] (+bias)

One agent per NeuronCore (A == 8 == n_cores), no collectives.

Layout: everything feature-major on chip (features on the 128
partitions, batch on the free dim).  The host pre-packs

    x   [128, 4096]     col = bt*1024 + k*512 + b  (bt-major batch tiles)
    wN  [128, K/128*M]  col = (k*mc + m)*128 + j   (k-chunk-major)
    bias[128, 11]       col j = 128-chunk j of [b1(4) b2(2) b3(4) b4(1)]

Data is bf16 (PSUM accumulates fp32; rel err ~5e-3, well inside the
2e-2 gate) so every input transfer is half the bytes of f32 and
LDWEIGHTS runs under FWL.  DMAs go out on three queues in
compute-need order: x on sync (HWDGE), w1/w2/w4 on scalar (HWDGE),
w3/bias on gpsimd (SWDGE) — so the two low-latency HWDGE queues carry
exactly what the first layer needs first.  Matmuls stream N=512
columns per instruction; the layers are emitted as a (bt + 2*layer)
diagonal wavefront so the in-order PE queue always has ready work.  A
couple of warm-up matmuls on a zeroed scratch tile bridge the gap
until the first x/w1 chunks land; the real layer-1 matmuls then run
through the tail of the HAM cold window (1.2 GHz) and everything after
runs at 2.4 GHz.  Post-matmul relu/copy is split across ScalarE and
VectorE with a fixed engine per destination tile; the final layer's
PSUM->SBUF copies alternate engines and its output DMAs alternate
queues, quartered on the last batch tile to shorten the drain.  The
kernel returns out^T [128, 2048] per core; the host transposes back.
"""

import os

import numpy as np

import concourse.bass as bass
import concourse.mybir as mybir
from concourse import bacc
from concourse.bass_utils import run_bass_kernel_spmd
from concourse.tile import TileContext

A, B, S = 8, 2048, 256
H1, H2, D1, D2 = 512, 256, 512, 128
P = 128
BT = 512            # batch tile (psum bank: 512 fp32)
NBT = B // BT

_DT_MAP = {
    "f32": mybir.dt.float32,
    "f32r": mybir.dt.float32r,
    "bf16": mybir.dt.bfloat16,
}

# layer: (k_chunks, m_chunks, bias col offset, relu?)
_LAYERS = [
    (S // P, H1 // P, 0, True),    # L1: 256 -> 512
    (H1 // P, H2 // P, 4, True),   # L2: 512 -> 256
    (H2 // P, D1 // P, 6, True),   # L3: 256 -> 512
    (D1 // P, D2 // P, 10, False), # L4: 512 -> 128
]


def _build(dt_name: str, add_bias: bool, warm: int) -> bass.Bass:
    dt = _DT_MAP[dt_name]
    f32 = mybir.dt.float32
    nc = bacc.Bacc(None, target_bir_lowering=False, debug=False)

    x_d = nc.dram_tensor("x", [P, (S // P) * B], dt, kind="ExternalInput")
    w_ds = [
        nc.dram_tensor("w1", [P, (S // P) * H1], dt, kind="ExternalInput"),
        nc.dram_tensor("w2", [P, (H1 // P) * H2], dt, kind="ExternalInput"),
        nc.dram_tensor("w3", [P, (H2 // P) * D1], dt, kind="ExternalInput"),
        nc.dram_tensor("w4", [P, (D1 // P) * D2], dt, kind="ExternalInput"),
    ]
    b_d = (
        nc.dram_tensor("bias", [P, 11], f32, kind="ExternalInput")
        if add_bias
        else None
    )
    out_d = nc.dram_tensor("out", [D2, B], f32, kind="ExternalOutput")

    with TileContext(nc) as tc:
        with (
            tc.tile_pool(name="persist", bufs=1) as pp,
            tc.tile_pool(name="psum", bufs=8, space="PSUM") as psp,
        ):
            xt = pp.tile([P, (S // P) * B], dt, tag="xt", name="xt")
            wts = [
                pp.tile(
                    [P, w_ds[i].shape[1]], dt, tag=f"w{i}", name=f"w{i}_sb"
                )
                for i in range(4)
            ]
            bti = (
                pp.tile([P, 11], f32, tag="bias", name="bias_sb")
                if add_bias
                else None
            )
            scr = (
                pp.tile([P, 2], f32, tag="scr", name="scr") if add_bias else None
            )
            acts = [
                [
                    pp.tile([P, B], dt, tag=f"a{li}_{i}", name=f"a{li}_{i}")
                    for i in range(n)
                ]
                for li, n in [(1, H1 // P), (2, H2 // P), (3, D1 // P)]
            ]
            acts.append([pp.tile([P, B], f32, tag="ot", name="ot")])

            # ---- PE warm-up: a couple of matmuls on a scratch tile keep
            # the PE busy (opening the HAM clock window) while the first
            # x / w1 DMA chunks are still in flight.  The scratch is
            # zeroed so no stray NaNs sit in the PE datapath.
            if warm > 0:
                wdt = f32 if dt == mybir.dt.float32r else dt
                wsb = pp.tile([P, BT], wdt, tag="wsb", name="wsb")
                nc.vector.memset(wsb[:], 0.0)
                wps = psp.tile([P, BT], f32, tag="ps", name="wps")
                lhs = wsb[:, 0:P]
                rhs = wsb[:]
                if dt == mybir.dt.float32r:
                    lhs = lhs.bitcast(dt)
                    rhs = rhs.bitcast(dt)
                for _ in range(warm):
                    nc.tensor.matmul(wps[:], lhs, rhs, start=True, stop=True)

            # ---- input DMAs, issued in compute-need order on the two
            # HWDGE queues.  x is host-packed bt-major (col = bt*2*BT +
            # k*BT + b) so each transfer is contiguous.  sync carries x
            # (first-layer critical path), scalar carries the weights in
            # the order the wavefront consumes them — exactly two queues
            # active at the start so the first x / w1 chunks get the full
            # SDMA packet bandwidth.
            kx = S // P

            def x_sl(bt, k, nk=1):
                return slice((bt * kx + k) * BT, (bt * kx + k + nk) * BT)

            # The first-needed pieces (x bt0-k0 + w1 k0) go out alone and
            # get the whole SDMA pipe; the bulk transfers are held back
            # until those land.  Without this the 16 SDMA engines
            # round-robin between the whole backlog of both queues and
            # the critical chunk's completion trickles in ~2us late.
            # The holds are sequencer-level stalls:
            #  - scalar: an in-queue copy READING the w1-k0 region picks
            #    up a wait on its DMA-completion sem, and every later
            #    dma_start on the scalar queue sits behind it.
            #  - sync has no compute ops, so a VectorE copy reads x00 and
            #    writes one column of x01's destination; the x01 DMA then
            #    carries a WAW wait on that copy (and the rest of the
            #    sync queue sits behind x01).
            gsc = pp.tile([P, 2], dt, tag="gate", name="gate_scratch")

            if add_bias:
                nc.scalar.dma_start(bti[:], b_d[:])
            nc.scalar.dma_start(wts[0][:, 0:128], w_ds[0][:, 0:128])
            nc.sync.dma_start(xt[:, x_sl(0, 0)], x_d[:, x_sl(0, 0)])
            nc.scalar.dma_start(wts[0][:, 128:512], w_ds[0][:, 128:512])
            # gates: stall each queue until its critical chunk landed
            x01 = x_sl(0, 1)
            nc.vector.tensor_copy(xt[:, x01.start : x01.start + 1], xt[:, 0:1])
            nc.scalar.copy(gsc[:, 1:2], wts[0][:, 0:1])
            nc.sync.dma_start(xt[:, x01], x_d[:, x01])
            nc.scalar.dma_start(wts[0][:, 512:1024], w_ds[0][:, 512:1024])
            nc.scalar.dma_start(wts[1][:], w_ds[1][:])
            for bt in range(1, NBT):
                nc.sync.dma_start(xt[:, x_sl(bt, 0, kx)], x_d[:, x_sl(bt, 0, kx)])
            nc.scalar.dma_start(wts[3][:], w_ds[3][:])
            nc.scalar.dma_start(wts[2][:], w_ds[2][:])
            if add_bias:
                # advance ACT/DVE engine clocks past the bias DMA so the
                # real post-matmul ops carry a single (PE) wait each — the
                # AC/DVE instruction structs have one wait slot.
                nc.scalar.copy(scr[:, 0:1], bti[:, 0:1])
                nc.vector.tensor_copy(scr[:, 1:2], bti[:, 0:1])

            # ---- the 4-layer chain, emitted as a (bt + 2*layer) diagonal
            # wavefront: the PE's in-order queue then always has ready
            # later-layer work to chew while L1 waits on x DMAs.
            def x_rhs(k, bt):
                return xt[:, (bt * kx + k) * BT : (bt * kx + k + 1) * BT]

            sched = sorted(
                ((bt + 2 * li, -li, bt) for li in range(4) for bt in range(NBT))
            )
            for _, nli, bt in sched:
                li = -nli
                kc, mc, boff, relu = _LAYERS[li]
                wt = wts[li]
                dsts = acts[li]
                srcs = acts[li - 1] if li > 0 else None
                if li == 0:
                    # k-outer for every L1 batch-tile: each k sweep needs
                    # only one x chunk + half of w1 in SBUF, so the
                    # supply-paced phase runs with fine-grained waits
                    pss = [
                        psp.tile([P, BT], f32, tag="ps", name=f"ps_l0_{bt}_{m}")
                        for m in range(mc)
                    ]
                    for k in range(kc):
                        for m in range(mc):
                            nc.tensor.matmul(
                                pss[m][:],
                                wt[:, (k * mc + m) * P : (k * mc + m + 1) * P],
                                x_rhs(k, bt),
                                start=(k == 0),
                                stop=(k == kc - 1),
                            )
                else:
                    pss = None
                for m in range(mc):
                    # fixed engine per dst tile: one writer per tile
                    use_act = (li < 3) and (m < mc // 2 or mc == 1)
                    if pss is not None:
                        ps = pss[m]
                    else:
                        ps = psp.tile([P, BT], f32, tag="ps", name="ps")
                        for k in range(kc):
                            rhs = (
                                x_rhs(k, bt)
                                if li == 0
                                else srcs[k][:, bt * BT : (bt + 1) * BT]
                            )
                            nc.tensor.matmul(
                                ps[:],
                                wt[:, (k * mc + m) * P : (k * mc + m + 1) * P],
                                rhs,
                                start=(k == 0),
                                stop=(k == kc - 1),
                            )
                    dst = dsts[m][:, bt * BT : (bt + 1) * BT]
                    if add_bias:
                        bias_ap = bti[:, boff + m : boff + m + 1]
                        if use_act:
                            func = (
                                mybir.ActivationFunctionType.Relu
                                if relu
                                else mybir.ActivationFunctionType.Identity
                            )
                            nc.scalar.activation(
                                dst, ps[:], func, bias=bias_ap
                            )
                        elif relu:
                            nc.vector.tensor_scalar(
                                dst,
                                ps[:],
                                bias_ap,
                                0.0,
                                mybir.AluOpType.add,
                                mybir.AluOpType.max,
                            )
                        elif li == 3:
                            # final layer: halve the copy across both
                            # engines so the out-DMA can start sooner
                            h = BT // 2
                            o = 0
                            nc.scalar.activation(
                                dst[:, o : o + h],
                                ps[:, o : o + h],
                                mybir.ActivationFunctionType.Identity,
                                bias=bias_ap,
                            )
                            nc.vector.tensor_scalar_add(
                                dst[:, h:], ps[:, h:], bias_ap
                            )
                        else:
                            nc.vector.tensor_scalar_add(dst, ps[:], bias_ap)
                    elif use_act:
                        func = (
                            mybir.ActivationFunctionType.Relu
                            if relu
                            else mybir.ActivationFunctionType.Copy
                        )
                        nc.scalar.activation(dst, ps[:], func)
                    elif relu:
                        nc.vector.tensor_scalar_max(dst, ps[:], 0.0)
                    else:
                        # final layer PSUM -> SBUF copy, split across
                        # ScalarE and VectorE; quartered on the last
                        # batch tile so the final out-DMA chunks are
                        # small and start early
                        nq = 4 if bt == NBT - 1 else 2
                        q = BT // nq
                        for j in range(nq):
                            csl = slice(j * q, (j + 1) * q)
                            if j % 2 == 0:
                                nc.scalar.activation(
                                    dst[:, csl],
                                    ps[:, csl],
                                    mybir.ActivationFunctionType.Copy,
                                )
                            else:
                                nc.vector.tensor_copy(dst[:, csl], ps[:, csl])
                if li == 3:
                    ot = acts[3][0]
                    if bt < NBT - 1:
                        eng = nc.sync if bt % 2 == 0 else nc.scalar
                        eng.dma_start(
                            out_d[:, bt * BT : (bt + 1) * BT],
                            ot[:, bt * BT : (bt + 1) * BT],
                        )
                    else:
                        # last tile: quarter across both queues to
                        # shorten the final drain
                        q = BT // 4
                        o = bt * BT
                        for j in range(4):
                            eng = nc.sync if j % 2 == 0 else nc.scalar
                            eng.dma_start(
                                out_d[:, o + j * q : o + (j + 1) * q],
                                ot[:, o + j * q : o + (j + 1) * q],
                            )
    nc.compile()
    return nc


_BUILT: dict[tuple, bass.Bass] = {}


def _cfg():
    dt_name = os.environ.get("MADPS_DT", "bf16")
    warm = int(os.environ.get("MADPS_WARM", "4"))
    return dt_name, warm


def _get_nc(dt_name: str, add_bias: bool, warm: int) -> bass.Bass:
    key = (dt_name, add_bias, warm)
    if key not in _BUILT:
        _BUILT[key] = _build(dt_name, add_bias, warm)
    return _BUILT[key]


def _np_dt(dt_name: str):
    if dt_name == "bf16":
        import ml_dtypes

        return ml_dtypes.bfloat16
    return np.float32


def _packw(w: np.ndarray, np_dt) -> np.ndarray:
    """[K, M] -> [128, (K/128)*M], k-chunk-major: col (k*mc + m)*128 + j."""
    k, m = w.shape
    kc = k // P
    return np.ascontiguousarray(
        w.reshape(kc, P, m).transpose(1, 0, 2).reshape(P, -1).astype(np_dt)
    )


def _prepare(inputs, dt_name):
    """Returns (add_bias, in_maps) for run_bass_kernel_spmd."""
    np_dt = _np_dt(dt_name)

    x = np.asarray(inputs["inputs"], dtype=np.float32)
    sel_s = np.asarray(inputs["laac_shallow"]).reshape(-1).astype(np.int64)
    sel_d = np.asarray(inputs["laac_deep"]).reshape(-1).astype(np.int64)
    Ws1 = np.asarray(inputs["Ws1"], dtype=np.float32)
    Ws2 = np.asarray(inputs["Ws2"], dtype=np.float32)
    Wd1 = np.asarray(inputs["Wd1"], dtype=np.float32)
    Wd2 = np.asarray(inputs["Wd2"], dtype=np.float32)
    bs1 = np.asarray(inputs["bs1"], dtype=np.float32)
    bs2 = np.asarray(inputs["bs2"], dtype=np.float32)
    bd1 = np.asarray(inputs["bd1"], dtype=np.float32)
    bd2 = np.asarray(inputs["bd2"], dtype=np.float32)

    add_bias = any(
        float(np.abs(b).max()) != 0.0 for b in (bs1, bs2, bd1, bd2)
    )

    in_maps = []
    for a in range(A):
        es, ed = int(sel_s[a]), int(sel_d[a])
        # bt-major packing: col = bt*(S//P)*BT + k*BT + b
        xp = np.ascontiguousarray(
            x[a]
            .reshape(NBT, BT, S // P, P)
            .transpose(3, 0, 2, 1)
            .reshape(P, -1)
            .astype(np_dt)
        )
        m = {
            "x": xp,
            "w1": _packw(Ws1[es], np_dt),
            "w2": _packw(Ws2[es], np_dt),
            "w3": _packw(Wd1[ed], np_dt),
            "w4": _packw(Wd2[ed], np_dt),
        }
        if add_bias:
            bias_cols = np.concatenate([bs1[es], bs2[es], bd1[ed], bd2[ed]])
            m["bias"] = np.ascontiguousarray(
                bias_cols.reshape(11, P).T, dtype=np.float32
            )
        in_maps.append(m)
    return add_bias, in_maps


def kernel(**inputs) -> np.ndarray:
    dt_name, warm = _cfg()
    add_bias, in_maps = _prepare(inputs, dt_name)
    nc = _get_nc(dt_name, add_bias, warm)
    res = run_bass_kernel_spmd(nc, in_maps, list(range(A)))
    out = np.stack([np.asarray(res.results[a]["out"]).T for a in range(A)])
    return np.ascontiguousarray(out.astype(np.float32))


# revision 15
# speedup vs baseline: 1.0169x; 1.0169x over previous
"""MADPSNet MoE-routing kernel for 8 Trainium2 NeuronCores.

The reference computes every expert on the full stacked input and then
gathers one expert per agent.  The routing indices (laac_shallow /
laac_deep) are host-visible numpy values, so we do the routing on the
host: per agent we select the 4 weight matrices of its chosen experts
and run only the selected chain

    x[2048,256] @ W1[256,512] -> relu -> @ W2[512,256] -> relu
                -> @ W3[256,512] -> relu -> @ W4[512,128] (+bias)

One agent per NeuronCore (A == 8 == n_cores), no collectives.

Layout: everything feature-major on chip (features on the 128
partitions, batch on the free dim).  The host pre-packs

    x   [128, 4096]     col = bt*1024 + k*512 + b  (bt-major batch tiles)
    wN  [128, K/128*M]  col = (k*mc + m)*128 + j   (k-chunk-major)
    bias[128, 11]       col j = 128-chunk j of [b1(4) b2(2) b3(4) b4(1)]

Data is bf16 (PSUM accumulates fp32; rel err ~5e-3, well inside the
2e-2 gate) so every input transfer is half the bytes of f32 and
LDWEIGHTS runs under FWL.  DMAs go out on three queues in
compute-need order: x on sync (HWDGE), w1/w2/w4 on scalar (HWDGE),
w3/bias on gpsimd (SWDGE) — so the two low-latency HWDGE queues carry
exactly what the first layer needs first.  Matmuls stream N=512
columns per instruction; the layers are emitted as a (bt + 2*layer)
diagonal wavefront so the in-order PE queue always has ready work.  A
couple of warm-up matmuls on a zeroed scratch tile bridge the gap
until the first x/w1 chunks land; the real layer-1 matmuls then run
through the tail of the HAM cold window (1.2 GHz) and everything after
runs at 2.4 GHz.  Post-matmul relu/copy is split across ScalarE and
VectorE with a fixed engine per destination tile; the final layer's
PSUM->SBUF copies alternate engines and its output DMAs alternate
queues, quartered on the last batch tile to shorten the drain.  The
kernel returns out^T [128, 2048] per core; the host transposes back.
"""

import os

import numpy as np

import concourse.bass as bass
import concourse.mybir as mybir
from concourse import bacc
from concourse.bass_utils import run_bass_kernel_spmd
from concourse.tile import TileContext

A, B, S = 8, 2048, 256
H1, H2, D1, D2 = 512, 256, 512, 128
P = 128
BT = 512            # batch tile (psum bank: 512 fp32)
NBT = B // BT

_DT_MAP = {
    "f32": mybir.dt.float32,
    "f32r": mybir.dt.float32r,
    "bf16": mybir.dt.bfloat16,
}

# layer: (k_chunks, m_chunks, bias col offset, relu?)
_LAYERS = [
    (S // P, H1 // P, 0, True),    # L1: 256 -> 512
    (H1 // P, H2 // P, 4, True),   # L2: 512 -> 256
    (H2 // P, D1 // P, 6, True),   # L3: 256 -> 512
    (D1 // P, D2 // P, 10, False), # L4: 512 -> 128
]


def _build(dt_name: str, add_bias: bool, warm: int) -> bass.Bass:
    dt = _DT_MAP[dt_name]
    f32 = mybir.dt.float32
    nc = bacc.Bacc(None, target_bir_lowering=False, debug=False)

    x_d = nc.dram_tensor("x", [P, (S // P) * B], dt, kind="ExternalInput")
    w_ds = [
        nc.dram_tensor("w1", [P, (S // P) * H1], dt, kind="ExternalInput"),
        nc.dram_tensor("w2", [P, (H1 // P) * H2], dt, kind="ExternalInput"),
        nc.dram_tensor("w3", [P, (H2 // P) * D1], dt, kind="ExternalInput"),
        nc.dram_tensor("w4", [P, (D1 // P) * D2], dt, kind="ExternalInput"),
    ]
    b_d = (
        nc.dram_tensor("bias", [P, 11], f32, kind="ExternalInput")
        if add_bias
        else None
    )
    out_d = nc.dram_tensor("out", [D2, B], f32, kind="ExternalOutput")

    with TileContext(nc) as tc:
        with (
            tc.tile_pool(name="persist", bufs=1) as pp,
            tc.tile_pool(name="psum", bufs=8, space="PSUM") as psp,
        ):
            xt = pp.tile([P, (S // P) * B], dt, tag="xt", name="xt")
            wts = [
                pp.tile(
                    [P, w_ds[i].shape[1]], dt, tag=f"w{i}", name=f"w{i}_sb"
                )
                for i in range(4)
            ]
            bti = (
                pp.tile([P, 11], f32, tag="bias", name="bias_sb")
                if add_bias
                else None
            )
            scr = (
                pp.tile([P, 2], f32, tag="scr", name="scr") if add_bias else None
            )
            acts = [
                [
                    pp.tile([P, B], dt, tag=f"a{li}_{i}", name=f"a{li}_{i}")
                    for i in range(n)
                ]
                for li, n in [(1, H1 // P), (2, H2 // P), (3, D1 // P)]
            ]
            acts.append([pp.tile([P, B], f32, tag="ot", name="ot")])

            # ---- PE warm-up: a couple of matmuls on a scratch tile keep
            # the PE busy (opening the HAM clock window) while the first
            # x / w1 DMA chunks are still in flight.  The scratch is
            # zeroed so no stray NaNs sit in the PE datapath.
            if warm > 0:
                wdt = f32 if dt == mybir.dt.float32r else dt
                wsb = pp.tile([P, BT], wdt, tag="wsb", name="wsb")
                nc.vector.memset(wsb[:], 0.0)
                wps = psp.tile([P, BT], f32, tag="ps", name="wps")
                lhs = wsb[:, 0:P]
                rhs = wsb[:]
                if dt == mybir.dt.float32r:
                    lhs = lhs.bitcast(dt)
                    rhs = rhs.bitcast(dt)
                for _ in range(warm):
                    nc.tensor.matmul(wps[:], lhs, rhs, start=True, stop=True)

            # ---- input DMAs, issued in compute-need order on the two
            # HWDGE queues.  x is host-packed bt-major (col = bt*2*BT +
            # k*BT + b) so each transfer is contiguous.  sync carries x
            # (first-layer critical path), scalar carries the weights in
            # the order the wavefront consumes them — exactly two queues
            # active at the start so the first x / w1 chunks get the full
            # SDMA packet bandwidth.
            kx = S // P

            def x_sl(bt, k, nk=1):
                return slice((bt * kx + k) * BT, (bt * kx + k + nk) * BT)

            # The completion sem of a DMA fires ~1-2us after its data
            # lands (HBM write-receipt round trip under full 8-core
            # load), so the first layer-1 matmul cannot retire before
            # ~11us no matter how the transfers are ordered; the warm-up
            # matmuls above are sized to bridge exactly that window.
            if add_bias:
                nc.scalar.dma_start(bti[:], b_d[:])
            nc.scalar.dma_start(wts[0][:, 0:128], w_ds[0][:, 0:128])
            nc.sync.dma_start(xt[:, x_sl(0, 0)], x_d[:, x_sl(0, 0)])
            nc.scalar.dma_start(wts[0][:, 128:512], w_ds[0][:, 128:512])
            nc.sync.dma_start(xt[:, x_sl(0, 1)], x_d[:, x_sl(0, 1)])
            nc.scalar.dma_start(wts[0][:, 512:1024], w_ds[0][:, 512:1024])
            nc.scalar.dma_start(wts[1][:], w_ds[1][:])
            for bt in range(1, NBT):
                nc.sync.dma_start(xt[:, x_sl(bt, 0, kx)], x_d[:, x_sl(bt, 0, kx)])
            nc.scalar.dma_start(wts[3][:], w_ds[3][:])
            nc.scalar.dma_start(wts[2][:], w_ds[2][:])
            if add_bias:
                # advance ACT/DVE engine clocks past the bias DMA so the
                # real post-matmul ops carry a single (PE) wait each — the
                # AC/DVE instruction structs have one wait slot.
                nc.scalar.copy(scr[:, 0:1], bti[:, 0:1])
                nc.vector.tensor_copy(scr[:, 1:2], bti[:, 0:1])

            # ---- the 4-layer chain, emitted as a (bt + 2*layer) diagonal
            # wavefront: the PE's in-order queue then always has ready
            # later-layer work to chew while L1 waits on x DMAs.
            def x_rhs(k, bt):
                return xt[:, (bt * kx + k) * BT : (bt * kx + k + 1) * BT]

            sched = sorted(
                ((bt + 2 * li, -li, bt) for li in range(4) for bt in range(NBT))
            )
            for _, nli, bt in sched:
                li = -nli
                kc, mc, boff, relu = _LAYERS[li]
                wt = wts[li]
                dsts = acts[li]
                srcs = acts[li - 1] if li > 0 else None
                if li == 0:
                    # k-outer for every L1 batch-tile: each k sweep needs
                    # only one x chunk + half of w1 in SBUF, so the
                    # supply-paced phase runs with fine-grained waits
                    pss = [
                        psp.tile([P, BT], f32, tag="ps", name=f"ps_l0_{bt}_{m}")
                        for m in range(mc)
                    ]
                    for k in range(kc):
                        for m in range(mc):
                            nc.tensor.matmul(
                                pss[m][:],
                                wt[:, (k * mc + m) * P : (k * mc + m + 1) * P],
                                x_rhs(k, bt),
                                start=(k == 0),
                                stop=(k == kc - 1),
                            )
                else:
                    pss = None
                for m in range(mc):
                    # fixed engine per dst tile: one writer per tile
                    use_act = (li < 3) and (m < mc // 2 or mc == 1)
                    if pss is not None:
                        ps = pss[m]
                    else:
                        ps = psp.tile([P, BT], f32, tag="ps", name="ps")
                        for k in range(kc):
                            rhs = (
                                x_rhs(k, bt)
                                if li == 0
                                else srcs[k][:, bt * BT : (bt + 1) * BT]
                            )
                            nc.tensor.matmul(
                                ps[:],
                                wt[:, (k * mc + m) * P : (k * mc + m + 1) * P],
                                rhs,
                                start=(k == 0),
                                stop=(k == kc - 1),
                            )
                    dst = dsts[m][:, bt * BT : (bt + 1) * BT]
                    if add_bias:
                        bias_ap = bti[:, boff + m : boff + m + 1]
                        if use_act:
                            func = (
                                mybir.ActivationFunctionType.Relu
                                if relu
                                else mybir.ActivationFunctionType.Identity
                            )
                            nc.scalar.activation(
                                dst, ps[:], func, bias=bias_ap
                            )
                        elif relu:
                            nc.vector.tensor_scalar(
                                dst,
                                ps[:],
                                bias_ap,
                                0.0,
                                mybir.AluOpType.add,
                                mybir.AluOpType.max,
                            )
                        elif li == 3:
                            # final layer: halve the copy across both
                            # engines so the out-DMA can start sooner
                            h = BT // 2
                            o = 0
                            nc.scalar.activation(
                                dst[:, o : o + h],
                                ps[:, o : o + h],
                                mybir.ActivationFunctionType.Identity,
                                bias=bias_ap,
                            )
                            nc.vector.tensor_scalar_add(
                                dst[:, h:], ps[:, h:], bias_ap
                            )
                        else:
                            nc.vector.tensor_scalar_add(dst, ps[:], bias_ap)
                    elif use_act:
                        func = (
                            mybir.ActivationFunctionType.Relu
                            if relu
                            else mybir.ActivationFunctionType.Copy
                        )
                        nc.scalar.activation(dst, ps[:], func)
                    elif relu:
                        nc.vector.tensor_scalar_max(dst, ps[:], 0.0)
                    else:
                        # final layer PSUM -> SBUF copy, split across
                        # ScalarE and VectorE; quartered on the last
                        # batch tile so the final out-DMA chunks are
                        # small and start early
                        nq = 4 if bt == NBT - 1 else 2
                        q = BT // nq
                        for j in range(nq):
                            csl = slice(j * q, (j + 1) * q)
                            if j % 2 == 0:
                                nc.scalar.activation(
                                    dst[:, csl],
                                    ps[:, csl],
                                    mybir.ActivationFunctionType.Copy,
                                )
                            else:
                                nc.vector.tensor_copy(dst[:, csl], ps[:, csl])
                if li == 3:
                    ot = acts[3][0]
                    if bt < NBT - 1:
                        eng = nc.sync if bt % 2 == 0 else nc.scalar
                        eng.dma_start(
                            out_d[:, bt * BT : (bt + 1) * BT],
                            ot[:, bt * BT : (bt + 1) * BT],
                        )
                    else:
                        # last tile: quarter across both queues to
                        # shorten the final drain
                        q = BT // 4
                        o = bt * BT
                        for j in range(4):
                            eng = nc.sync if j % 2 == 0 else nc.scalar
                            eng.dma_start(
                                out_d[:, o + j * q : o + (j + 1) * q],
                                ot[:, o + j * q : o + (j + 1) * q],
                            )
    nc.compile()
    return nc


_BUILT: dict[tuple, bass.Bass] = {}


def _cfg():
    dt_name = os.environ.get("MADPS_DT", "bf16")
    warm = int(os.environ.get("MADPS_WARM", "8"))
    return dt_name, warm


def _get_nc(dt_name: str, add_bias: bool, warm: int) -> bass.Bass:
    key = (dt_name, add_bias, warm)
    if key not in _BUILT:
        _BUILT[key] = _build(dt_name, add_bias, warm)
    return _BUILT[key]


def _np_dt(dt_name: str):
    if dt_name == "bf16":
        import ml_dtypes

        return ml_dtypes.bfloat16
    return np.float32


def _packw(w: np.ndarray, np_dt) -> np.ndarray:
    """[K, M] -> [128, (K/128)*M], k-chunk-major: col (k*mc + m)*128 + j."""
    k, m = w.shape
    kc = k // P
    return np.ascontiguousarray(
        w.reshape(kc, P, m).transpose(1, 0, 2).reshape(P, -1).astype(np_dt)
    )


def _prepare(inputs, dt_name):
    """Returns (add_bias, in_maps) for run_bass_kernel_spmd."""
    np_dt = _np_dt(dt_name)

    x = np.asarray(inputs["inputs"], dtype=np.float32)
    sel_s = np.asarray(inputs["laac_shallow"]).reshape(-1).astype(np.int64)
    sel_d = np.asarray(inputs["laac_deep"]).reshape(-1).astype(np.int64)
    Ws1 = np.asarray(inputs["Ws1"], dtype=np.float32)
    Ws2 = np.asarray(inputs["Ws2"], dtype=np.float32)
    Wd1 = np.asarray(inputs["Wd1"], dtype=np.float32)
    Wd2 = np.asarray(inputs["Wd2"], dtype=np.float32)
    bs1 = np.asarray(inputs["bs1"], dtype=np.float32)
    bs2 = np.asarray(inputs["bs2"], dtype=np.float32)
    bd1 = np.asarray(inputs["bd1"], dtype=np.float32)
    bd2 = np.asarray(inputs["bd2"], dtype=np.float32)

    add_bias = any(
        float(np.abs(b).max()) != 0.0 for b in (bs1, bs2, bd1, bd2)
    )

    in_maps = []
    for a in range(A):
        es, ed = int(sel_s[a]), int(sel_d[a])
        # bt-major packing: col = bt*(S//P)*BT + k*BT + b
        xp = np.ascontiguousarray(
            x[a]
            .reshape(NBT, BT, S // P, P)
            .transpose(3, 0, 2, 1)
            .reshape(P, -1)
            .astype(np_dt)
        )
        m = {
            "x": xp,
            "w1": _packw(Ws1[es], np_dt),
            "w2": _packw(Ws2[es], np_dt),
            "w3": _packw(Wd1[ed], np_dt),
            "w4": _packw(Wd2[ed], np_dt),
        }
        if add_bias:
            bias_cols = np.concatenate([bs1[es], bs2[es], bd1[ed], bd2[ed]])
            m["bias"] = np.ascontiguousarray(
                bias_cols.reshape(11, P).T, dtype=np.float32
            )
        in_maps.append(m)
    return add_bias, in_maps


def kernel(**inputs) -> np.ndarray:
    dt_name, warm = _cfg()
    add_bias, in_maps = _prepare(inputs, dt_name)
    nc = _get_nc(dt_name, add_bias, warm)
    res = run_bass_kernel_spmd(nc, in_maps, list(range(A)))
    out = np.stack([np.asarray(res.results[a]["out"]).T for a in range(A)])
    return np.ascontiguousarray(out.astype(np.float32))


# revision 17
# speedup vs baseline: 1.0531x; 1.0356x over previous
"""MADPSNet MoE-routing kernel for 8 Trainium2 NeuronCores.

The reference computes every expert on the full stacked input and then
gathers one expert per agent.  The routing indices (laac_shallow /
laac_deep) are host-visible numpy values, so we do the routing on the
host: per agent we select the 4 weight matrices of its chosen experts
and run only the selected chain

    x[2048,256] @ W1[256,512] -> relu -> @ W2[512,256] -> relu
                -> @ W3[256,512] -> relu -> @ W4[512,128] (+bias)

One agent per NeuronCore (A == 8 == n_cores), no collectives.

Layout: everything feature-major on chip (features on the 128
partitions, batch on the free dim).  The host pre-packs

    x   [128, 4096]     col = bt*1024 + k*512 + b  (bt-major batch tiles)
    wN  [128, K/128*M]  col = (k*mc + m)*128 + j   (k-chunk-major)
    bias[128, 11]       col j = 128-chunk j of [b1(4) b2(2) b3(4) b4(1)]

Data is bf16 (PSUM accumulates fp32; rel err ~5e-3, well inside the
2e-2 gate) so every input transfer is half the bytes of f32 and
LDWEIGHTS runs under FWL.  DMAs go out on three queues in
compute-need order: x on sync (HWDGE), w1/w2/w4 on scalar (HWDGE),
w3/bias on gpsimd (SWDGE) — so the two low-latency HWDGE queues carry
exactly what the first layer needs first.  Matmuls stream N=512
columns per instruction; the layers are emitted as a (bt + 2*layer)
diagonal wavefront so the in-order PE queue always has ready work.  A
couple of warm-up matmuls on a zeroed scratch tile bridge the gap
until the first x/w1 chunks land; the real layer-1 matmuls then run
through the tail of the HAM cold window (1.2 GHz) and everything after
runs at 2.4 GHz.  Post-matmul relu/copy is split across ScalarE and
VectorE with a fixed engine per destination tile; the final layer's
PSUM->SBUF copies alternate engines and its output DMAs alternate
queues, quartered on the last batch tile to shorten the drain.  The
kernel returns out^T [128, 2048] per core; the host transposes back.
"""

import os

import numpy as np

import concourse.bass as bass
import concourse.mybir as mybir
from concourse import bacc
from concourse.bass_utils import run_bass_kernel_spmd
from concourse.tile import TileContext

A, B, S = 8, 2048, 256
H1, H2, D1, D2 = 512, 256, 512, 128
P = 128
BT = 512            # batch tile (psum bank: 512 fp32)
NBT = B // BT

_DT_MAP = {
    "f32": mybir.dt.float32,
    "f32r": mybir.dt.float32r,
    "bf16": mybir.dt.bfloat16,
}

# layer: (k_chunks, m_chunks, bias col offset, relu?)
_LAYERS = [
    (S // P, H1 // P, 0, True),    # L1: 256 -> 512
    (H1 // P, H2 // P, 4, True),   # L2: 512 -> 256
    (H2 // P, D1 // P, 6, True),   # L3: 256 -> 512
    (D1 // P, D2 // P, 10, False), # L4: 512 -> 128
]


def _build(dt_name: str, add_bias: bool, warm: int) -> bass.Bass:
    dt = _DT_MAP[dt_name]
    f32 = mybir.dt.float32
    nc = bacc.Bacc(None, target_bir_lowering=False, debug=False)

    x_d = nc.dram_tensor("x", [P, (S // P) * B], dt, kind="ExternalInput")
    w_ds = [
        nc.dram_tensor("w1", [P, (S // P) * H1], dt, kind="ExternalInput"),
        nc.dram_tensor("w2", [P, (H1 // P) * H2], dt, kind="ExternalInput"),
        nc.dram_tensor("w3", [P, (H2 // P) * D1], dt, kind="ExternalInput"),
        nc.dram_tensor("w4", [P, (D1 // P) * D2], dt, kind="ExternalInput"),
    ]
    b_d = (
        nc.dram_tensor("bias", [P, 11], f32, kind="ExternalInput")
        if add_bias
        else None
    )
    out_d = nc.dram_tensor("out", [D2, B], f32, kind="ExternalOutput")

    with TileContext(nc) as tc:
        with (
            tc.tile_pool(name="persist", bufs=1) as pp,
            tc.tile_pool(name="psum", bufs=8, space="PSUM") as psp,
        ):
            xt = pp.tile([P, (S // P) * B], dt, tag="xt", name="xt")
            wts = [
                pp.tile(
                    [P, w_ds[i].shape[1]], dt, tag=f"w{i}", name=f"w{i}_sb"
                )
                for i in range(4)
            ]
            bti = (
                pp.tile([P, 11], f32, tag="bias", name="bias_sb")
                if add_bias
                else None
            )
            scr = (
                pp.tile([P, 2], f32, tag="scr", name="scr") if add_bias else None
            )
            acts = [
                [
                    pp.tile([P, B], dt, tag=f"a{li}_{i}", name=f"a{li}_{i}")
                    for i in range(n)
                ]
                for li, n in [(1, H1 // P), (2, H2 // P), (3, D1 // P)]
            ]
            acts.append([pp.tile([P, B], f32, tag="ot", name="ot")])

            # ---- PE warm-up: a couple of matmuls on a scratch tile keep
            # the PE busy (opening the HAM clock window) while the first
            # x / w1 DMA chunks are still in flight.  The scratch is
            # zeroed so no stray NaNs sit in the PE datapath.
            if warm > 0:
                wdt = f32 if dt == mybir.dt.float32r else dt
                wsb = pp.tile([P, BT], wdt, tag="wsb", name="wsb")
                nc.vector.memset(wsb[:], 0.0)
                wps = psp.tile([P, BT], f32, tag="ps", name="wps")
                lhs = wsb[:, 0:P]
                rhs = wsb[:]
                if dt == mybir.dt.float32r:
                    lhs = lhs.bitcast(dt)
                    rhs = rhs.bitcast(dt)
                for _ in range(warm):
                    nc.tensor.matmul(wps[:], lhs, rhs, start=True, stop=True)

            # ---- input DMAs, issued in compute-need order on the two
            # HWDGE queues.  x is host-packed bt-major (col = bt*2*BT +
            # k*BT + b) so each transfer is contiguous.  sync carries x
            # (first-layer critical path), scalar carries the weights in
            # the order the wavefront consumes them — exactly two queues
            # active at the start so the first x / w1 chunks get the full
            # SDMA packet bandwidth.
            kx = S // P

            def x_sl(bt, k, nk=1):
                return slice((bt * kx + k) * BT, (bt * kx + k + nk) * BT)

            # The completion sem of a DMA fires ~1-2us after its data
            # lands (HBM write-receipt round trip under full 8-core
            # load), so the first layer-1 matmul cannot retire before
            # ~11us no matter how the transfers are ordered; the warm-up
            # matmuls above are sized to bridge exactly that window.
            # x moves per batch-tile (256KB) so each tile's k0+k1 share
            # one completion sem — the L1 k-sweep never waits on a
            # separate k1 transfer mid-accumulation, and the ~1us
            # inter-bt sem spacing is faster than the ~1.7us per-bt
            # compute, so the stream stays fed.
            if add_bias:
                nc.scalar.dma_start(bti[:], b_d[:])
            nc.scalar.dma_start(wts[0][:, 0:512], w_ds[0][:, 0:512])
            for bt in range(NBT):
                nc.sync.dma_start(xt[:, x_sl(bt, 0, kx)], x_d[:, x_sl(bt, 0, kx)])
            nc.scalar.dma_start(wts[0][:, 512:1024], w_ds[0][:, 512:1024])
            nc.scalar.dma_start(wts[1][:], w_ds[1][:])
            nc.scalar.dma_start(wts[3][:], w_ds[3][:])
            nc.scalar.dma_start(wts[2][:], w_ds[2][:])
            if add_bias:
                # advance ACT/DVE engine clocks past the bias DMA so the
                # real post-matmul ops carry a single (PE) wait each — the
                # AC/DVE instruction structs have one wait slot.
                nc.scalar.copy(scr[:, 0:1], bti[:, 0:1])
                nc.vector.tensor_copy(scr[:, 1:2], bti[:, 0:1])

            # ---- the 4-layer chain, emitted as a (bt + 2*layer) diagonal
            # wavefront: the PE's in-order queue then always has ready
            # later-layer work to chew while L1 waits on x DMAs.
            def x_rhs(k, bt):
                return xt[:, (bt * kx + k) * BT : (bt * kx + k + 1) * BT]

            sched = sorted(
                ((bt + 2 * li, -li, bt) for li in range(4) for bt in range(NBT))
            )
            for _, nli, bt in sched:
                li = -nli
                kc, mc, boff, relu = _LAYERS[li]
                wt = wts[li]
                dsts = acts[li]
                srcs = acts[li - 1] if li > 0 else None
                if li == 0:
                    # k-outer for every L1 batch-tile: each k sweep needs
                    # only one x chunk + half of w1 in SBUF, so the
                    # supply-paced phase runs with fine-grained waits
                    pss = [
                        psp.tile([P, BT], f32, tag="ps", name=f"ps_l0_{bt}_{m}")
                        for m in range(mc)
                    ]
                    for k in range(kc):
                        for m in range(mc):
                            nc.tensor.matmul(
                                pss[m][:],
                                wt[:, (k * mc + m) * P : (k * mc + m + 1) * P],
                                x_rhs(k, bt),
                                start=(k == 0),
                                stop=(k == kc - 1),
                            )
                else:
                    pss = None
                for m in range(mc):
                    # fixed engine per dst tile: one writer per tile
                    use_act = (li < 3) and (m < mc // 2 or mc == 1)
                    if pss is not None:
                        ps = pss[m]
                    else:
                        ps = psp.tile([P, BT], f32, tag="ps", name="ps")
                        for k in range(kc):
                            rhs = (
                                x_rhs(k, bt)
                                if li == 0
                                else srcs[k][:, bt * BT : (bt + 1) * BT]
                            )
                            nc.tensor.matmul(
                                ps[:],
                                wt[:, (k * mc + m) * P : (k * mc + m + 1) * P],
                                rhs,
                                start=(k == 0),
                                stop=(k == kc - 1),
                            )
                    dst = dsts[m][:, bt * BT : (bt + 1) * BT]
                    if add_bias:
                        bias_ap = bti[:, boff + m : boff + m + 1]
                        if use_act:
                            func = (
                                mybir.ActivationFunctionType.Relu
                                if relu
                                else mybir.ActivationFunctionType.Identity
                            )
                            nc.scalar.activation(
                                dst, ps[:], func, bias=bias_ap
                            )
                        elif relu:
                            nc.vector.tensor_scalar(
                                dst,
                                ps[:],
                                bias_ap,
                                0.0,
                                mybir.AluOpType.add,
                                mybir.AluOpType.max,
                            )
                        elif li == 3:
                            # final layer: halve the copy across both
                            # engines so the out-DMA can start sooner
                            h = BT // 2
                            o = 0
                            nc.scalar.activation(
                                dst[:, o : o + h],
                                ps[:, o : o + h],
                                mybir.ActivationFunctionType.Identity,
                                bias=bias_ap,
                            )
                            nc.vector.tensor_scalar_add(
                                dst[:, h:], ps[:, h:], bias_ap
                            )
                        else:
                            nc.vector.tensor_scalar_add(dst, ps[:], bias_ap)
                    elif use_act:
                        func = (
                            mybir.ActivationFunctionType.Relu
                            if relu
                            else mybir.ActivationFunctionType.Copy
                        )
                        nc.scalar.activation(dst, ps[:], func)
                    elif relu:
                        nc.vector.tensor_scalar_max(dst, ps[:], 0.0)
                    else:
                        # final layer PSUM -> SBUF copy, split across
                        # ScalarE and VectorE; quartered on the last
                        # batch tile so the final out-DMA chunks are
                        # small and start early
                        nq = 4 if bt == NBT - 1 else 2
                        q = BT // nq
                        for j in range(nq):
                            csl = slice(j * q, (j + 1) * q)
                            if j % 2 == 0:
                                nc.scalar.activation(
                                    dst[:, csl],
                                    ps[:, csl],
                                    mybir.ActivationFunctionType.Copy,
                                )
                            else:
                                nc.vector.tensor_copy(dst[:, csl], ps[:, csl])
                if li == 3:
                    ot = acts[3][0]
                    if bt < NBT - 1:
                        eng = nc.sync if bt % 2 == 0 else nc.scalar
                        eng.dma_start(
                            out_d[:, bt * BT : (bt + 1) * BT],
                            ot[:, bt * BT : (bt + 1) * BT],
                        )
                    else:
                        # last tile: quarter across both queues to
                        # shorten the final drain
                        q = BT // 4
                        o = bt * BT
                        for j in range(4):
                            eng = nc.sync if j % 2 == 0 else nc.scalar
                            eng.dma_start(
                                out_d[:, o + j * q : o + (j + 1) * q],
                                ot[:, o + j * q : o + (j + 1) * q],
                            )
    nc.compile()
    return nc


_BUILT: dict[tuple, bass.Bass] = {}


def _cfg():
    dt_name = os.environ.get("MADPS_DT", "bf16")
    warm = int(os.environ.get("MADPS_WARM", "9"))
    return dt_name, warm


def _get_nc(dt_name: str, add_bias: bool, warm: int) -> bass.Bass:
    key = (dt_name, add_bias, warm)
    if key not in _BUILT:
        _BUILT[key] = _build(dt_name, add_bias, warm)
    return _BUILT[key]


def _np_dt(dt_name: str):
    if dt_name == "bf16":
        import ml_dtypes

        return ml_dtypes.bfloat16
    return np.float32


def _packw(w: np.ndarray, np_dt) -> np.ndarray:
    """[K, M] -> [128, (K/128)*M], k-chunk-major: col (k*mc + m)*128 + j."""
    k, m = w.shape
    kc = k // P
    return np.ascontiguousarray(
        w.reshape(kc, P, m).transpose(1, 0, 2).reshape(P, -1).astype(np_dt)
    )


def _prepare(inputs, dt_name):
    """Returns (add_bias, in_maps) for run_bass_kernel_spmd."""
    np_dt = _np_dt(dt_name)

    x = np.asarray(inputs["inputs"], dtype=np.float32)
    sel_s = np.asarray(inputs["laac_shallow"]).reshape(-1).astype(np.int64)
    sel_d = np.asarray(inputs["laac_deep"]).reshape(-1).astype(np.int64)
    Ws1 = np.asarray(inputs["Ws1"], dtype=np.float32)
    Ws2 = np.asarray(inputs["Ws2"], dtype=np.float32)
    Wd1 = np.asarray(inputs["Wd1"], dtype=np.float32)
    Wd2 = np.asarray(inputs["Wd2"], dtype=np.float32)
    bs1 = np.asarray(inputs["bs1"], dtype=np.float32)
    bs2 = np.asarray(inputs["bs2"], dtype=np.float32)
    bd1 = np.asarray(inputs["bd1"], dtype=np.float32)
    bd2 = np.asarray(inputs["bd2"], dtype=np.float32)

    add_bias = any(
        float(np.abs(b).max()) != 0.0 for b in (bs1, bs2, bd1, bd2)
    )

    in_maps = []
    for a in range(A):
        es, ed = int(sel_s[a]), int(sel_d[a])
        # bt-major packing: col = bt*(S//P)*BT + k*BT + b
        xp = np.ascontiguousarray(
            x[a]
            .reshape(NBT, BT, S // P, P)
            .transpose(3, 0, 2, 1)
            .reshape(P, -1)
            .astype(np_dt)
        )
        m = {
            "x": xp,
            "w1": _packw(Ws1[es], np_dt),
            "w2": _packw(Ws2[es], np_dt),
            "w3": _packw(Wd1[ed], np_dt),
            "w4": _packw(Wd2[ed], np_dt),
        }
        if add_bias:
            bias_cols = np.concatenate([bs1[es], bs2[es], bd1[ed], bd2[ed]])
            m["bias"] = np.ascontiguousarray(
                bias_cols.reshape(11, P).T, dtype=np.float32
            )
        in_maps.append(m)
    return add_bias, in_maps


def kernel(**inputs) -> np.ndarray:
    dt_name, warm = _cfg()
    add_bias, in_maps = _prepare(inputs, dt_name)
    nc = _get_nc(dt_name, add_bias, warm)
    res = run_bass_kernel_spmd(nc, in_maps, list(range(A)))
    out = np.stack([np.asarray(res.results[a]["out"]).T for a in range(A)])
    return np.ascontiguousarray(out.astype(np.float32))
